# revision 34
# baseline (speedup 1.0000x reference)
"""Branching-Kriging pairwise kernel matrix on 8 Trainium2 NeuronCores.

Math: for rows i of W1 and j of W2,
    K(i,j) = exp(share_k + branch_k + nested_k)
Every term is a sum over products of a function of i and a function of j
(the categorical branch/level structure is one-hot encodable), so
    log K = F1 @ F2.T
with F1 [4096, 79] and F2 [2048, 79] feature matrices (padded to 128).
The device kernel is a K=128 fp16 matmul + ACT exp (fp16 out) + 16 MiB
output write, sharded along n1 (rows of W1) across the 8 cores.

Timing notes (from perfetto traces): the measured exec window opens at
the first *engine* instruction — the unconditional Bass-preamble GpSimd
MEMSETs at ~6.4us — and closes at the end of the runtime epilogue
(all-engine semaphore sweep, ~8.3us after the last engine program
ends). The on-clock critical path is: input DMA (16 shared rings,
~15-22GB/s each; fp16 halves the bytes) -> matmuls (ACT chases the PE
column-wise, so exps start with their matmul's ISSUE) -> 8 serialized
ACT exps (the steady-state bottleneck, ~1.0us per [128,1024] chunk) ->
epilogue. Output DMAs and their tail are fully hidden under the
epilogue sweep (no completion wait), so only the exp stream's end
matters. Some runs see ~1.2x engine-clock throttling (DMA timing is
unaffected); every cross-engine race here is margined for that.
"""

import numpy as np

import concourse.bass as bass
import concourse.mybir as mybir
from concourse.bass_utils import run_bass_kernel_spmd

N_CORES = 8
N1, N2 = 4096, 2048
ROWS = N1 // N_CORES          # 512 output rows per core
D = 128                       # feature (contraction) dim, padded from 79
S, B = 8, 3                   # spatial / branching factor counts
NEST = [3, 3, 3]              # nested factors per branching factor

FP32 = mybir.dt.float32
FP16 = mybir.dt.float16


def _act(x):
    return np.minimum(np.where(x >= 0.0, x + 1.0, np.exp(x)), 30.0).astype(np.float32)


def _build_features(W1, W2, alpha, theta, gamma0, gamma1, gamma2):
    """log K = F1 @ F2.T, exactly (up to fp16 operand rounding)."""
    W1 = np.asarray(W1, np.float32)
    W2 = np.asarray(W2, np.float32)
    n1, n2 = W1.shape[0], W2.shape[0]
    X1, Z1, V1 = W1[:, :S], W1[:, S:S + B], W1[:, S + B:]
    X2, Z2, V2 = W2[:, :S], W2[:, S:S + B], W2[:, S + B:]
    a = _act(np.asarray(alpha))[0]            # [S]
    t = _act(np.asarray(theta))[0]            # [B]
    G = [_act(np.asarray(g)) - 1.0 for g in (gamma0, gamma1, gamma2)]  # [nb, 4]

    F1 = np.zeros((n1, D), np.float32)
    F2 = np.zeros((n2, D), np.float32)

    # row terms + constant
    F1[:, 0] = 1.0
    F2[:, 0] = -(X2**2 @ a) - (V2**2).sum(1) - t.sum()
    F1[:, 1] = -(X1**2 @ a) - (V1**2).sum(1)
    F2[:, 1] = 1.0
    # share cross: 2 a_s x1 x2
    F1[:, 2:10] = 2.0 * a[None, :] * X1
    F2[:, 2:10] = X2
    # nested v cross (level-independent part): 2 v1 v2
    F1[:, 10:19] = 2.0 * V1
    F2[:, 10:19] = V2

    d = 19
    Z1i = Z1.astype(np.int32)
    Z2i = Z2.astype(np.int32)
    off = 0
    for b in range(B):
        nb = NEST[b]
        v1b = V1[:, off:off + nb]
        v2b = V2[:, off:off + nb]
        for lev in range(1, 5):
            e1 = (Z1i[:, b] == lev).astype(np.float32)
            e2 = (Z2i[:, b] == lev).astype(np.float32)
            g = G[b][:, lev - 1]
            # branch match reward t_b, minus gamma-weighted v2^2
            F1[:, d] = e1
            F2[:, d] = e2 * (t[b] - (v2b**2) @ g)
            d += 1
            # gamma-weighted v1^2
            F1[:, d] = -e1 * ((v1b**2) @ g)
            F2[:, d] = e2
            d += 1
            # gamma-weighted cross terms
            F1[:, d:d + nb] = 2.0 * e1[:, None] * v1b * g[None, :]
            F2[:, d:d + nb] = e2[:, None] * v2b
            d += nb
        off += nb
    assert d == 79

    # Operands go to the PE as fp16 (F1, the stationary side) and bf16
    # (F2, the moving side — bf16 streams through the PE at full rate
    # where fp16 runs half-rate). Pre-round both feature matrices on the
    # host, then spend the spare contraction dims (79..127) on
    # residual-correction columns for the worst error contributors:
    # F*G = r(F)r(G) + L_F r(G) + r(F) L_G up to a negligible L_F*L_G term.
    def _r16(x):
        return np.float32(np.float16(x))

    def _rbf(x):
        b = np.ascontiguousarray(np.float32(x)).view(np.uint32)
        return ((b + 0x7FFF + ((b >> 16) & 1)) & 0xFFFF0000).view(np.float32)

    nd = d
    L1 = F1[:, :nd] - _r16(F1[:, :nd])
    L2 = F2[:, :nd] - _rbf(F2[:, :nd])
    c1 = np.abs(L1).max(0) * np.abs(F2[:, :nd]).max(0)
    c2 = np.abs(F1[:, :nd]).max(0) * np.abs(L2).max(0)
    cand = [(c1[i], i, 1) for i in range(nd)] + [(c2[i], i, 2) for i in range(nd)]
    cand.sort(key=lambda t: -t[0])
    F1[:, :nd] = _r16(F1[:, :nd])
    F2[:, :nd] = _rbf(F2[:, :nd])
    for c, i, side in cand[:D - nd]:
        if c <= 0.0:
            break
        if side == 1:
            F1[:, d] = _r16(L1[:, i])
            F2[:, d] = F2[:, i]
        else:
            F1[:, d] = F1[:, i]
            F2[:, d] = _rbf(L2[:, i])
        d += 1
    # F2 as raw bf16 bit patterns viewed as uint16 (packed into the fp16
    # input buffer; the device bitcasts the view back to bf16)
    f2bits = (np.ascontiguousarray(F2).view(np.uint32) >> 16).astype(np.uint16)
    return np.float16(F1), f2bits


_COMPILED = None


def _get_nc():
    """Raw Bass program (no TileContext): hand-placed semaphores, no
    end-of-kernel butterfly barriers or semaphore-sweep from Tile.

    Per core: load F1-shard.T [128,512] fp16 + F2.T [128,2048] bf16
    (three pipelined DMAs on sync, pre-Block), 16 matmuls into four
    2-bank [128,1024] PSUM tiles in column-half-major order, 8
    [128,1024] exps on ACT writing fp16, 8 output DMAs from sync with
    the last two pre-fired behind a junk padding transfer, all
    software-pipelined.
    """
    global _COMPILED
    if _COMPILED is not None:
        return _COMPILED

    nc = bass.Bass(target_bir_lowering=False, debug=False)
    # single packed input [f1_shard.T | f2.T] fp16: long (3-5KB/partition)
    # descriptors for full input bandwidth
    fin = nc.dram_tensor("fin", [D, ROWS + N2], FP16, kind="ExternalInput")
    out = nc.dram_tensor("out", [ROWS, N2], FP16, kind="ExternalOutput")
    junk = nc.dram_tensor("junk", [128, N2 // 2], FP16, kind="Internal")

    MT = ROWS // 128          # 4 output row-blocks per core
    H = N2 // 2               # 1024: half-width exp/store granularity
    EXPF = mybir.ActivationFunctionType.Exp

    with (
        nc.sbuf_tensor("fins", [D, ROWS + N2], FP16) as fins,
        nc.sbuf_tensor("ots", [128, 8 * H], FP16) as ots,
        nc.psum_tensor("ps0", [128, N2 // 2], FP32) as ps0,
        nc.psum_tensor("ps1", [128, N2 // 2], FP32) as ps1,
        nc.psum_tensor("ps2", [128, N2 // 2], FP32) as ps2,
        nc.psum_tensor("ps3", [128, N2 // 2], FP32) as ps3,
        nc.semaphore("in_a_sem") as in_a_sem,
        nc.semaphore("in_b_sem") as in_b_sem,
        nc.semaphore("in_c_sem") as in_c_sem,
        nc.semaphore("in_d_sem") as in_d_sem,
        nc.semaphore("mm_sem") as mm_sem,
        nc.semaphore("act_sem") as act_sem,
        nc.semaphore("out_sem") as out_sem,
    ):
        pss = [ps0, ps1, ps2, ps3]

        # packed input layout (host packs in the same order):
        #   [f1 block0 (128) | f2[0:512] | f2[512:1024] | f1 blocks1-3
        #    (384) | f2[1024:2048]]
        # so each of the four input DMAs is a CONTIGUOUS slice carrying
        # exactly the operands the next pipeline stage unblocks on:
        # d_a -> mm0 (f1b0 + its f2 chunk, only 640 cols, lands ~0.4us
        # earlier than a 1024-col split), d_b -> mm1's f2 chunk,
        # d_c -> f1 blocks 1-3 (needed one exp-chunk later), d_d -> f2's
        # second half (not touched until the h1 chunks, ~4us later).
        # SEPARATE semaphores per DMA (a single staged semaphore races:
        # fast rings can contribute several increments while slow rings
        # are still on the first sub-DMA).
        def w1(mt):
            # f1 row-block mt inside the packed sbuf tile
            if mt == 0:
                return fins[:, 0:128]
            return fins[:, 1152 + (mt - 1) * 128:1152 + mt * 128]

        def f2col(c):
            # f2^T column block c; bf16 bit patterns packed host-side
            # into the fp16 buffer (bitcast back on device)
            o = 128 + c if c < 1024 else 1536 + (c - 1024)
            return fins[:, o:o + 512].bitcast(mybir.dt.bfloat16)

        nc.sync.dma_start(fins[:, 0:640], fin[:, 0:640]).then_inc(in_a_sem, 16)
        nc.sync.dma_start(fins[:, 640:1152], fin[:, 640:1152]).then_inc(in_b_sem, 16)
        nc.sync.dma_start(fins[:, 1152:1536], fin[:, 1152:1536]).then_inc(in_c_sem, 16)
        nc.sync.dma_start(fins[:, 1536:2560], fin[:, 1536:2560]).then_inc(in_d_sem, 16)

        # chunk j = h*MT + mt  ->  psum tile pss[mt], out block (mt, h);
        # the first half-tile (mt0,h0) is further split into two 512-wide
        # exp/store chunks gated on mm0/mm1 individually, so the exp
        # stream starts one matmul earlier. echunks: (mt, col0, width).
        # echunk entries: (mt, out_col, psum_col, width)
        chunks = [(mt, h) for h in range(2) for mt in range(MT)]
        echunks = [(0, 0, 0, 512), (0, 512, 512, 512)] + [
            (mt, h * H, 0, H) for h in range(2) for mt in range(MT)
        ][1:]
        NCH = len(echunks)     # 9

        with nc.Block(no_gpsimd_drain=True) as block:
            @block.scalar
            def _(scalar):
                # 1024-wide dummy reading (garbage) PSUM: loads the ACT
                # exp table during the input transfer in the SAME basic
                # block as the real exps (walrus' table-state pass resets
                # at BB boundaries and would re-load, +1.3us). It finishes
                # well before the first matmul writes ps0, and its garbage
                # output in ots chunk 0 is fully overwritten by the real
                # chunk-0 exp before any DMA reads it.
                nc.scalar.activation(ots[:, 0:512], ps0[:, 0:512], EXPF)
                o = 0
                for j, (mt, c0, pc, w) in enumerate(echunks):
                    scalar.wait_ge(mm_sem, j + 1)
                    nc.scalar.activation(
                        ots[:, o:o + w],
                        pss[mt][:, pc:pc + w],
                        EXPF,
                    ).then_inc(act_sem)
                    o += w

            @block.tensor
            def _(tensor):
                for _w in range(3):
                    nc.tensor.matmul(ps3[:, 0:512], w1(3), f2col(1024),
                                     start=True, stop=True)
                tensor.wait_ge(in_a_sem, 16)
                for j, (mt, h) in enumerate(chunks):
                    ps = pss[mt]
                    if j == 1:
                        tensor.wait_ge(in_c_sem, 16)
                    if j == MT:
                        tensor.wait_ge(in_d_sem, 16)
                    if h == 1:
                        # reuse pss[mt]: wait until its h0 exps read out
                        # (echunk indices shift +1 from the mt0 split)
                        tensor.wait_ge(act_sem, mt + 2)
                    c = h * H
                    mma = nc.tensor.matmul(ps[:, 0:512], w1(mt), f2col(c),
                                           start=True, stop=True)
                    if j == 0:
                        mma.then_inc(mm_sem)
                        tensor.wait_ge(in_b_sem, 16)
                    nc.tensor.matmul(ps[:, 512:1024], w1(mt), f2col(c + 512),
                                     start=True, stop=True).then_inc(mm_sem)

            @block.sync
            def _(sync):
                # out_sem is incremented (DGE requires sync info) but never
                # waited on: the runtime epilogue (~8us all-engine sem
                # sweep after every program ends) hides the whole output
                # tail, so only sync's LAST INSTRUCTION time matters. The
                # trailing triggers pre-fire one exp early (waits below) so
                # the ~0.6us-per-DIRECT2D issue chain finishes before the
                # final exp retires; data safety comes from ring FIFO
                # order — each pre-fired chunk's descriptors queue behind
                # earlier chunks plus the junk padding transfer, so their
                # data is read only after their exp completes, with >=0.8us
                # margin that holds under engine-clock throttling (which
                # stretches the gating exps as much as the issue chain).
                waits = [1, 2, 3, 4, 5, 5, 6, 7, 7]
                o = 0
                for j, (mt, c0, pc, w) in enumerate(echunks):
                    sync.wait_ge(act_sem, waits[j])
                    sync.dma_start(
                        out[mt * 128:(mt + 1) * 128, c0:c0 + w],
                        ots[:, o:o + w],
                    ).then_inc(out_sem, 16)
                    o += w
                    if j == 6:
                        sync.dma_start(
                            junk[:, :], ots[:, 0:N2 // 2],
                        ).then_inc(out_sem, 16)

    # no explicit end-of-kernel semaphore cleanup: the NEFF's runtime
    # epilogue already sweeps every HW semaphore back to 0 on each engine
    # (observed as the anonymous $S[n]=0 EVENT_SEMAPHORE waves in traces),
    # so a re-execution of the loaded NEFF starts clean regardless

    _COMPILED = nc
    return _COMPILED


LAST_RESULTS = None


def _ensure_ntff_hook():
    """The agent image's `antenv` lacks `axon_hooks`; register the
    boot-shipped ctypes NTFF hook under that name so trace=True works."""
    import sys
    import types

    try:
        import antenv.axon_hooks  # noqa: F401
        return
    except ImportError:
        pass
    mod = types.ModuleType("antenv.axon_hooks")
    mod._hook = None

    def set_axon_ntff_profile_hook(hook):
        mod._hook = hook

    def get_axon_ntff_profile_hook():
        return mod._hook

    mod.set_axon_ntff_profile_hook = set_axon_ntff_profile_hook
    mod.get_axon_ntff_profile_hook = get_axon_ntff_profile_hook
    sys.modules["antenv.axon_hooks"] = mod
    import antenv

    antenv.axon_hooks = mod
    try:
        from trn_agent_boot.trn_boot import _ntff_profile_via_ctypes

        mod._hook = _ntff_profile_via_ctypes("/opt/axon/libaxon_pjrt.so")
    except Exception:
        pass
    # artifact upload needs bucket creds this container may not have;
    # the local NTFF -> perfetto pipeline doesn't depend on it
    import concourse.bass_utils as _bu

    _orig_upload = _bu.upload_artifacts

    def _safe_upload(tmpdir):
        try:
            return _orig_upload(tmpdir)
        except Exception:
            return tmpdir

    _bu.upload_artifacts = _safe_upload


def kernel(W1, W2, alpha, theta, gamma0, gamma1, gamma2, _profile=False):
    global LAST_RESULTS
    if _profile:
        _ensure_ntff_hook()
    F1, F2bits = _build_features(W1, W2, alpha, theta, gamma0, gamma1, gamma2)
    f1t = np.ascontiguousarray(F1.T)      # [D, N1] fp16
    # [D, N2] bf16 bit patterns reinterpreted as fp16 for the packed buffer
    f2t = np.ascontiguousarray(F2bits.T).view(np.float16)
    # packed per-core layout (must match the device's w1/f2col mapping):
    # [f1 block0 | f2[0:512] | f2[512:1024] | f1 blocks1-3 | f2[1024:2048]]
    in_maps = []
    for c in range(N_CORES):
        sh = f1t[:, c * ROWS:(c + 1) * ROWS]
        in_maps.append({
            "fin": np.ascontiguousarray(np.concatenate(
                [sh[:, 0:128], f2t[:, 0:512], f2t[:, 512:1024],
                 sh[:, 128:512], f2t[:, 1024:2048]], axis=1)),
        })
    nc = _get_nc()
    res = run_bass_kernel_spmd(nc, in_maps, list(range(N_CORES)), trace=_profile)
    LAST_RESULTS = res
    return np.concatenate(
        [res.results[c]["out"] for c in range(N_CORES)], axis=0
    ).astype(np.float32)


# revision 35
# speedup vs baseline: 1.0279x; 1.0279x over previous
"""Branching-Kriging pairwise kernel matrix on 8 Trainium2 NeuronCores.

Math: for rows i of W1 and j of W2,
    K(i,j) = exp(share_k + branch_k + nested_k)
Every term is a sum over products of a function of i and a function of j
(the categorical branch/level structure is one-hot encodable), so
    log K = F1 @ F2.T
with F1 [4096, 79] and F2 [2048, 79] feature matrices (padded to 128).
The device kernel is a K=128 fp16 matmul + ACT exp (fp16 out) + 16 MiB
output write, sharded along n1 (rows of W1) across the 8 cores.

Timing notes (from perfetto traces): the measured exec window opens at
the first *engine* instruction — the unconditional Bass-preamble GpSimd
MEMSETs at ~6.4us — and closes at the end of the runtime epilogue
(all-engine semaphore sweep, ~8.3us after the last engine program
ends). The on-clock critical path is: input DMA (16 shared rings,
~15-22GB/s each; fp16 halves the bytes) -> matmuls (ACT chases the PE
column-wise, so exps start with their matmul's ISSUE) -> 8 serialized
ACT exps (the steady-state bottleneck, ~1.0us per [128,1024] chunk) ->
epilogue. Output DMAs and their tail are fully hidden under the
epilogue sweep (no completion wait), so only the exp stream's end
matters. Some runs see ~1.2x engine-clock throttling (DMA timing is
unaffected); every cross-engine race here is margined for that.
"""

import numpy as np

import concourse.bass as bass
import concourse.mybir as mybir
from concourse.bass_utils import run_bass_kernel_spmd

N_CORES = 8
N1, N2 = 4096, 2048
ROWS = N1 // N_CORES          # 512 output rows per core
D = 128                       # feature (contraction) dim, padded from 79
S, B = 8, 3                   # spatial / branching factor counts
NEST = [3, 3, 3]              # nested factors per branching factor

FP32 = mybir.dt.float32
FP16 = mybir.dt.float16


def _act(x):
    return np.minimum(np.where(x >= 0.0, x + 1.0, np.exp(x)), 30.0).astype(np.float32)


def _build_features(W1, W2, alpha, theta, gamma0, gamma1, gamma2):
    """log K = F1 @ F2.T, exactly (up to fp16 operand rounding)."""
    W1 = np.asarray(W1, np.float32)
    W2 = np.asarray(W2, np.float32)
    n1, n2 = W1.shape[0], W2.shape[0]
    X1, Z1, V1 = W1[:, :S], W1[:, S:S + B], W1[:, S + B:]
    X2, Z2, V2 = W2[:, :S], W2[:, S:S + B], W2[:, S + B:]
    a = _act(np.asarray(alpha))[0]            # [S]
    t = _act(np.asarray(theta))[0]            # [B]
    G = [_act(np.asarray(g)) - 1.0 for g in (gamma0, gamma1, gamma2)]  # [nb, 4]

    F1 = np.zeros((n1, D), np.float32)
    F2 = np.zeros((n2, D), np.float32)

    # row terms + constant
    F1[:, 0] = 1.0
    F2[:, 0] = -(X2**2 @ a) - (V2**2).sum(1) - t.sum()
    F1[:, 1] = -(X1**2 @ a) - (V1**2).sum(1)
    F2[:, 1] = 1.0
    # share cross: 2 a_s x1 x2
    F1[:, 2:10] = 2.0 * a[None, :] * X1
    F2[:, 2:10] = X2
    # nested v cross (level-independent part): 2 v1 v2
    F1[:, 10:19] = 2.0 * V1
    F2[:, 10:19] = V2

    d = 19
    Z1i = Z1.astype(np.int32)
    Z2i = Z2.astype(np.int32)
    off = 0
    for b in range(B):
        nb = NEST[b]
        v1b = V1[:, off:off + nb]
        v2b = V2[:, off:off + nb]
        for lev in range(1, 5):
            e1 = (Z1i[:, b] == lev).astype(np.float32)
            e2 = (Z2i[:, b] == lev).astype(np.float32)
            g = G[b][:, lev - 1]
            # branch match reward t_b, minus gamma-weighted v2^2
            F1[:, d] = e1
            F2[:, d] = e2 * (t[b] - (v2b**2) @ g)
            d += 1
            # gamma-weighted v1^2
            F1[:, d] = -e1 * ((v1b**2) @ g)
            F2[:, d] = e2
            d += 1
            # gamma-weighted cross terms
            F1[:, d:d + nb] = 2.0 * e1[:, None] * v1b * g[None, :]
            F2[:, d:d + nb] = e2[:, None] * v2b
            d += nb
        off += nb
    assert d == 79

    # Operands go to the PE as fp16 (F1, the stationary side) and bf16
    # (F2, the moving side — bf16 streams through the PE at full rate
    # where fp16 runs half-rate). Pre-round both feature matrices on the
    # host, then spend the spare contraction dims (79..127) on
    # residual-correction columns for the worst error contributors:
    # F*G = r(F)r(G) + L_F r(G) + r(F) L_G up to a negligible L_F*L_G term.
    def _r16(x):
        return np.float32(np.float16(x))

    def _rbf(x):
        b = np.ascontiguousarray(np.float32(x)).view(np.uint32)
        return ((b + 0x7FFF + ((b >> 16) & 1)) & 0xFFFF0000).view(np.float32)

    nd = d
    L1 = F1[:, :nd] - _r16(F1[:, :nd])
    L2 = F2[:, :nd] - _rbf(F2[:, :nd])
    c1 = np.abs(L1).max(0) * np.abs(F2[:, :nd]).max(0)
    c2 = np.abs(F1[:, :nd]).max(0) * np.abs(L2).max(0)
    cand = [(c1[i], i, 1) for i in range(nd)] + [(c2[i], i, 2) for i in range(nd)]
    cand.sort(key=lambda t: -t[0])
    F1[:, :nd] = _r16(F1[:, :nd])
    F2[:, :nd] = _rbf(F2[:, :nd])
    for c, i, side in cand[:D - nd]:
        if c <= 0.0:
            break
        if side == 1:
            F1[:, d] = _r16(L1[:, i])
            F2[:, d] = F2[:, i]
        else:
            F1[:, d] = F1[:, i]
            F2[:, d] = _rbf(L2[:, i])
        d += 1
    # F2 as raw bf16 bit patterns viewed as uint16 (packed into the fp16
    # input buffer; the device bitcasts the view back to bf16)
    f2bits = (np.ascontiguousarray(F2).view(np.uint32) >> 16).astype(np.uint16)
    return np.float16(F1), f2bits


_COMPILED = None


def _get_nc():
    """Raw Bass program (no TileContext): hand-placed semaphores, no
    end-of-kernel butterfly barriers or semaphore-sweep from Tile.

    Per core: load F1-shard.T [128,512] fp16 + F2.T [128,2048] bf16
    (three pipelined DMAs on sync, pre-Block), 16 matmuls into four
    2-bank [128,1024] PSUM tiles in column-half-major order, 8
    [128,1024] exps on ACT writing fp16, 8 output DMAs from sync with
    the last two pre-fired behind a junk padding transfer, all
    software-pipelined.
    """
    global _COMPILED
    if _COMPILED is not None:
        return _COMPILED

    nc = bass.Bass(target_bir_lowering=False, debug=False)
    # single packed input [f1_shard.T | f2.T] fp16: long (3-5KB/partition)
    # descriptors for full input bandwidth
    fin = nc.dram_tensor("fin", [D, ROWS + N2], FP16, kind="ExternalInput")
    out = nc.dram_tensor("out", [ROWS, N2], FP16, kind="ExternalOutput")
    junk = nc.dram_tensor("junk", [128, N2 // 2], FP16, kind="Internal")

    MT = ROWS // 128          # 4 output row-blocks per core
    H = N2 // 2               # 1024: half-width exp/store granularity
    EXPF = mybir.ActivationFunctionType.Exp

    with (
        nc.sbuf_tensor("fins", [D, ROWS + N2], FP16) as fins,
        nc.sbuf_tensor("ots", [128, 8 * H], FP16) as ots,
        nc.psum_tensor("ps0", [128, N2 // 2], FP32) as ps0,
        nc.psum_tensor("ps1", [128, N2 // 2], FP32) as ps1,
        nc.psum_tensor("ps2", [128, N2 // 2], FP32) as ps2,
        nc.psum_tensor("ps3", [128, N2 // 2], FP32) as ps3,
        nc.semaphore("in_a_sem") as in_a_sem,
        nc.semaphore("in_b_sem") as in_b_sem,
        nc.semaphore("in_c_sem") as in_c_sem,
        nc.semaphore("in_d_sem") as in_d_sem,
        nc.semaphore("mm_sem") as mm_sem,
        nc.semaphore("act_sem") as act_sem,
        nc.semaphore("out_sem") as out_sem,
    ):
        pss = [ps0, ps1, ps2, ps3]

        # packed input layout (host packs in the same order):
        #   [f1 block0 (128) | f2[0:512] | f2[512:1024] | f1 blocks1-3
        #    (384) | f2[1024:2048]]
        # so each of the four input DMAs is a CONTIGUOUS slice carrying
        # exactly the operands the next pipeline stage unblocks on:
        # d_a -> mm0 (f1b0 + its f2 chunk, only 640 cols, lands ~0.4us
        # earlier than a 1024-col split), d_b -> mm1's f2 chunk,
        # d_c -> f1 blocks 1-3 (needed one exp-chunk later), d_d -> f2's
        # second half (not touched until the h1 chunks, ~4us later).
        # SEPARATE semaphores per DMA (a single staged semaphore races:
        # fast rings can contribute several increments while slow rings
        # are still on the first sub-DMA).
        def w1(mt):
            # f1 row-block mt inside the packed sbuf tile
            if mt == 0:
                return fins[:, 0:128]
            return fins[:, 1152 + (mt - 1) * 128:1152 + mt * 128]

        def f2col(c):
            # f2^T column block c; bf16 bit patterns packed host-side
            # into the fp16 buffer (bitcast back on device)
            o = 128 + c if c < 1024 else 1536 + (c - 1024)
            return fins[:, o:o + 512].bitcast(mybir.dt.bfloat16)

        nc.sync.dma_start(fins[:, 0:640], fin[:, 0:640]).then_inc(in_a_sem, 16)
        nc.sync.dma_start(fins[:, 640:1152], fin[:, 640:1152]).then_inc(in_b_sem, 16)
        nc.sync.dma_start(fins[:, 1152:1536], fin[:, 1152:1536]).then_inc(in_c_sem, 16)
        nc.sync.dma_start(fins[:, 1536:2560], fin[:, 1536:2560]).then_inc(in_d_sem, 16)

        # chunk j = h*MT + mt  ->  psum tile pss[mt], out block (mt, h);
        # the first half-tile (mt0,h0) is further split into two 512-wide
        # exp/store chunks gated on mm0/mm1 individually, so the exp
        # stream starts one matmul earlier. echunks: (mt, col0, width).
        # echunk entries: (mt, out_col, psum_col, width)
        chunks = [(mt, h) for h in range(2) for mt in range(MT)]
        echunks = [(0, 0, 0, 512), (0, 512, 512, 512)] + [
            (mt, h * H, 0, H) for h in range(2) for mt in range(MT)
        ][1:]
        NCH = len(echunks)     # 9

        with nc.Block() as block:
            @block.scalar
            def _(scalar):
                # 1024-wide dummy reading (garbage) PSUM: loads the ACT
                # exp table during the input transfer in the SAME basic
                # block as the real exps (walrus' table-state pass resets
                # at BB boundaries and would re-load, +1.3us). It finishes
                # well before the first matmul writes ps0, and its garbage
                # output in ots chunk 0 is fully overwritten by the real
                # chunk-0 exp before any DMA reads it.
                nc.scalar.activation(ots[:, 0:H], ps0[:, :], EXPF)
                o = 0
                for j, (mt, c0, pc, w) in enumerate(echunks):
                    scalar.wait_ge(mm_sem, j + 1)
                    nc.scalar.activation(
                        ots[:, o:o + w],
                        pss[mt][:, pc:pc + w],
                        EXPF,
                    ).then_inc(act_sem)
                    o += w

            @block.tensor
            def _(tensor):
                tensor.wait_ge(in_a_sem, 16)
                for j, (mt, h) in enumerate(chunks):
                    ps = pss[mt]
                    if j == 1:
                        tensor.wait_ge(in_c_sem, 16)
                    if j == MT:
                        tensor.wait_ge(in_d_sem, 16)
                    if h == 1:
                        # reuse pss[mt]: wait until its h0 exps read out
                        # (echunk indices shift +1 from the mt0 split)
                        tensor.wait_ge(act_sem, mt + 2)
                    c = h * H
                    mma = nc.tensor.matmul(ps[:, 0:512], w1(mt), f2col(c),
                                           start=True, stop=True)
                    if j == 0:
                        mma.then_inc(mm_sem)
                        tensor.wait_ge(in_b_sem, 16)
                    nc.tensor.matmul(ps[:, 512:1024], w1(mt), f2col(c + 512),
                                     start=True, stop=True).then_inc(mm_sem)

            @block.sync
            def _(sync):
                # out_sem is incremented (DGE requires sync info) but never
                # waited on: the runtime epilogue (~8us all-engine sem
                # sweep after every program ends) hides the whole output
                # tail, so only sync's LAST INSTRUCTION time matters. The
                # trailing triggers pre-fire one exp early (waits below) so
                # the ~0.6us-per-DIRECT2D issue chain finishes before the
                # final exp retires; data safety comes from ring FIFO
                # order — each pre-fired chunk's descriptors queue behind
                # earlier chunks plus the junk padding transfer, so their
                # data is read only after their exp completes, with >=0.8us
                # margin that holds under engine-clock throttling (which
                # stretches the gating exps as much as the issue chain).
                waits = [1, 2, 3, 4, 5, 5, 6, 7, 7]
                o = 0
                for j, (mt, c0, pc, w) in enumerate(echunks):
                    sync.wait_ge(act_sem, waits[j])
                    sync.dma_start(
                        out[mt * 128:(mt + 1) * 128, c0:c0 + w],
                        ots[:, o:o + w],
                    ).then_inc(out_sem, 16)
                    o += w
                    if j == 6:
                        sync.dma_start(
                            junk[:, :], ots[:, 0:N2 // 2],
                        ).then_inc(out_sem, 16)

    # no explicit end-of-kernel semaphore cleanup: the NEFF's runtime
    # epilogue already sweeps every HW semaphore back to 0 on each engine
    # (observed as the anonymous $S[n]=0 EVENT_SEMAPHORE waves in traces),
    # so a re-execution of the loaded NEFF starts clean regardless

    _COMPILED = nc
    return _COMPILED


LAST_RESULTS = None


def _ensure_ntff_hook():
    """The agent image's `antenv` lacks `axon_hooks`; register the
    boot-shipped ctypes NTFF hook under that name so trace=True works."""
    import sys
    import types

    try:
        import antenv.axon_hooks  # noqa: F401
        return
    except ImportError:
        pass
    mod = types.ModuleType("antenv.axon_hooks")
    mod._hook = None

    def set_axon_ntff_profile_hook(hook):
        mod._hook = hook

    def get_axon_ntff_profile_hook():
        return mod._hook

    mod.set_axon_ntff_profile_hook = set_axon_ntff_profile_hook
    mod.get_axon_ntff_profile_hook = get_axon_ntff_profile_hook
    sys.modules["antenv.axon_hooks"] = mod
    import antenv

    antenv.axon_hooks = mod
    try:
        from trn_agent_boot.trn_boot import _ntff_profile_via_ctypes

        mod._hook = _ntff_profile_via_ctypes("/opt/axon/libaxon_pjrt.so")
    except Exception:
        pass
    # artifact upload needs bucket creds this container may not have;
    # the local NTFF -> perfetto pipeline doesn't depend on it
    import concourse.bass_utils as _bu

    _orig_upload = _bu.upload_artifacts

    def _safe_upload(tmpdir):
        try:
            return _orig_upload(tmpdir)
        except Exception:
            return tmpdir

    _bu.upload_artifacts = _safe_upload


def kernel(W1, W2, alpha, theta, gamma0, gamma1, gamma2, _profile=False):
    global LAST_RESULTS
    if _profile:
        _ensure_ntff_hook()
    F1, F2bits = _build_features(W1, W2, alpha, theta, gamma0, gamma1, gamma2)
    f1t = np.ascontiguousarray(F1.T)      # [D, N1] fp16
    # [D, N2] bf16 bit patterns reinterpreted as fp16 for the packed buffer
    f2t = np.ascontiguousarray(F2bits.T).view(np.float16)
    # packed per-core layout (must match the device's w1/f2col mapping):
    # [f1 block0 | f2[0:512] | f2[512:1024] | f1 blocks1-3 | f2[1024:2048]]
    in_maps = []
    for c in range(N_CORES):
        sh = f1t[:, c * ROWS:(c + 1) * ROWS]
        in_maps.append({
            "fin": np.ascontiguousarray(np.concatenate(
                [sh[:, 0:128], f2t[:, 0:512], f2t[:, 512:1024],
                 sh[:, 128:512], f2t[:, 1024:2048]], axis=1)),
        })
    nc = _get_nc()
    res = run_bass_kernel_spmd(nc, in_maps, list(range(N_CORES)), trace=_profile)
    LAST_RESULTS = res
    return np.concatenate(
        [res.results[c]["out"] for c in range(N_CORES)], axis=0
    ).astype(np.float32)


# revision 36
# speedup vs baseline: 1.0510x; 1.0225x over previous
"""Branching-Kriging pairwise kernel matrix on 8 Trainium2 NeuronCores.

Math: for rows i of W1 and j of W2,
    K(i,j) = exp(share_k + branch_k + nested_k)
Every term is a sum over products of a function of i and a function of j
(the categorical branch/level structure is one-hot encodable), so
    log K = F1 @ F2.T
with F1 [4096, 79] and F2 [2048, 79] feature matrices (padded to 128).
The device kernel is a K=128 fp16 matmul + ACT exp (fp16 out) + 16 MiB
output write, sharded along n1 (rows of W1) across the 8 cores.

Timing notes (from perfetto traces): the measured exec window opens at
the first *engine* instruction — the unconditional Bass-preamble GpSimd
MEMSETs at ~6.4us — and closes at the end of the runtime epilogue
(all-engine semaphore sweep, ~8.3us after the last engine program
ends). The on-clock critical path is: input DMA (16 shared rings,
~15-22GB/s each; fp16 halves the bytes) -> matmuls (ACT chases the PE
column-wise, so exps start with their matmul's ISSUE) -> 8 serialized
ACT exps (the steady-state bottleneck, ~1.0us per [128,1024] chunk) ->
epilogue. Output DMAs and their tail are fully hidden under the
epilogue sweep (no completion wait), so only the exp stream's end
matters. Some runs see ~1.2x engine-clock throttling (DMA timing is
unaffected); every cross-engine race here is margined for that.
"""

import numpy as np

import concourse.bass as bass
import concourse.mybir as mybir
from concourse.bass_utils import run_bass_kernel_spmd

N_CORES = 8
N1, N2 = 4096, 2048
ROWS = N1 // N_CORES          # 512 output rows per core
D = 128                       # feature (contraction) dim, padded from 79
S, B = 8, 3                   # spatial / branching factor counts
NEST = [3, 3, 3]              # nested factors per branching factor

FP32 = mybir.dt.float32
FP16 = mybir.dt.float16


def _act(x):
    return np.minimum(np.where(x >= 0.0, x + 1.0, np.exp(x)), 30.0).astype(np.float32)


def _build_features(W1, W2, alpha, theta, gamma0, gamma1, gamma2):
    """log K = F1 @ F2.T, exactly (up to fp16 operand rounding)."""
    W1 = np.asarray(W1, np.float32)
    W2 = np.asarray(W2, np.float32)
    n1, n2 = W1.shape[0], W2.shape[0]
    X1, Z1, V1 = W1[:, :S], W1[:, S:S + B], W1[:, S + B:]
    X2, Z2, V2 = W2[:, :S], W2[:, S:S + B], W2[:, S + B:]
    a = _act(np.asarray(alpha))[0]            # [S]
    t = _act(np.asarray(theta))[0]            # [B]
    G = [_act(np.asarray(g)) - 1.0 for g in (gamma0, gamma1, gamma2)]  # [nb, 4]

    F1 = np.zeros((n1, D), np.float32)
    F2 = np.zeros((n2, D), np.float32)

    # row terms + constant
    F1[:, 0] = 1.0
    F2[:, 0] = -(X2**2 @ a) - (V2**2).sum(1) - t.sum()
    F1[:, 1] = -(X1**2 @ a) - (V1**2).sum(1)
    F2[:, 1] = 1.0
    # share cross: 2 a_s x1 x2
    F1[:, 2:10] = 2.0 * a[None, :] * X1
    F2[:, 2:10] = X2
    # nested v cross (level-independent part): 2 v1 v2
    F1[:, 10:19] = 2.0 * V1
    F2[:, 10:19] = V2

    d = 19
    Z1i = Z1.astype(np.int32)
    Z2i = Z2.astype(np.int32)
    off = 0
    for b in range(B):
        nb = NEST[b]
        v1b = V1[:, off:off + nb]
        v2b = V2[:, off:off + nb]
        for lev in range(1, 5):
            e1 = (Z1i[:, b] == lev).astype(np.float32)
            e2 = (Z2i[:, b] == lev).astype(np.float32)
            g = G[b][:, lev - 1]
            # branch match reward t_b, minus gamma-weighted v2^2
            F1[:, d] = e1
            F2[:, d] = e2 * (t[b] - (v2b**2) @ g)
            d += 1
            # gamma-weighted v1^2
            F1[:, d] = -e1 * ((v1b**2) @ g)
            F2[:, d] = e2
            d += 1
            # gamma-weighted cross terms
            F1[:, d:d + nb] = 2.0 * e1[:, None] * v1b * g[None, :]
            F2[:, d:d + nb] = e2[:, None] * v2b
            d += nb
        off += nb
    assert d == 79

    # Operands go to the PE as fp16 (F1, the stationary side) and bf16
    # (F2, the moving side — bf16 streams through the PE at full rate
    # where fp16 runs half-rate). Pre-round both feature matrices on the
    # host, then spend the spare contraction dims (79..127) on
    # residual-correction columns for the worst error contributors:
    # F*G = r(F)r(G) + L_F r(G) + r(F) L_G up to a negligible L_F*L_G term.
    def _r16(x):
        return np.float32(np.float16(x))

    def _rbf(x):
        b = np.ascontiguousarray(np.float32(x)).view(np.uint32)
        return ((b + 0x7FFF + ((b >> 16) & 1)) & 0xFFFF0000).view(np.float32)

    nd = d
    L1 = F1[:, :nd] - _r16(F1[:, :nd])
    L2 = F2[:, :nd] - _rbf(F2[:, :nd])
    c1 = np.abs(L1).max(0) * np.abs(F2[:, :nd]).max(0)
    c2 = np.abs(F1[:, :nd]).max(0) * np.abs(L2).max(0)
    cand = [(c1[i], i, 1) for i in range(nd)] + [(c2[i], i, 2) for i in range(nd)]
    cand.sort(key=lambda t: -t[0])
    F1[:, :nd] = _r16(F1[:, :nd])
    F2[:, :nd] = _rbf(F2[:, :nd])
    for c, i, side in cand[:D - nd]:
        if c <= 0.0:
            break
        if side == 1:
            F1[:, d] = _r16(L1[:, i])
            F2[:, d] = F2[:, i]
        else:
            F1[:, d] = F1[:, i]
            F2[:, d] = _rbf(L2[:, i])
        d += 1
    # F2 as raw bf16 bit patterns viewed as uint16 (packed into the fp16
    # input buffer; the device bitcasts the view back to bf16)
    f2bits = (np.ascontiguousarray(F2).view(np.uint32) >> 16).astype(np.uint16)
    return np.float16(F1), f2bits


_COMPILED = None


def _get_nc():
    """Raw Bass program (no TileContext): hand-placed semaphores, no
    end-of-kernel butterfly barriers or semaphore-sweep from Tile.

    Per core: load F1-shard.T [128,512] fp16 + F2.T [128,2048] bf16
    (four pipelined pipeline-ordered DMAs on sync, pre-Block), 16
    matmuls into four 2-bank [128,1024] PSUM tiles in column-half-major
    order, 9 exps on ACT writing fp16 (first half-tile split 512+512 to
    start one matmul earlier), 9 output DMAs from sync with the
    trailing triggers pre-fired behind a junk padding transfer, all
    software-pipelined.
    """
    global _COMPILED
    if _COMPILED is not None:
        return _COMPILED

    nc = bass.Bass(target_bir_lowering=False, debug=False)
    # packed input [f1b0 | f2a | f2b | f1b1-3 | f2c] (see layout comment
    # below), 2-byte elements throughout
    fin = nc.dram_tensor("fin", [D, ROWS + N2], FP16, kind="ExternalInput")
    out = nc.dram_tensor("out", [ROWS, N2], FP16, kind="ExternalOutput")
    junk = nc.dram_tensor("junk", [128, N2 // 2], FP16, kind="Internal")

    MT = ROWS // 128          # 4 output row-blocks per core
    H = N2 // 2               # 1024: half-width exp/store granularity
    EXPF = mybir.ActivationFunctionType.Exp

    with (
        nc.sbuf_tensor("fins", [D, ROWS + N2], FP16) as fins,
        nc.sbuf_tensor("ots", [128, 8 * H], FP16) as ots,
        nc.psum_tensor("ps0", [128, N2 // 2], FP32) as ps0,
        nc.psum_tensor("ps1", [128, N2 // 2], FP32) as ps1,
        nc.psum_tensor("ps2", [128, N2 // 2], FP32) as ps2,
        nc.psum_tensor("ps3", [128, N2 // 2], FP32) as ps3,
        nc.semaphore("in_a_sem") as in_a_sem,
        nc.semaphore("in_b_sem") as in_b_sem,
        nc.semaphore("in_c_sem") as in_c_sem,
        nc.semaphore("in_d_sem") as in_d_sem,
        nc.semaphore("mm_sem") as mm_sem,
        nc.semaphore("act_sem") as act_sem,
        nc.semaphore("out_sem") as out_sem,
    ):
        pss = [ps0, ps1, ps2, ps3]

        # packed input layout (host packs in the same order):
        #   [f1 block0 (128) | f2[0:512] | f2[512:1024] | f1 blocks1-3
        #    (384) | f2[1024:2048]]
        # so each of the four input DMAs is a CONTIGUOUS slice carrying
        # exactly the operands the next pipeline stage unblocks on:
        # d_a -> mm0 (f1b0 + its f2 chunk, only 640 cols, lands ~0.4us
        # earlier than a 1024-col split), d_b -> mm1's f2 chunk,
        # d_c -> f1 blocks 1-3 (needed one exp-chunk later), d_d -> f2's
        # second half (not touched until the h1 chunks, ~4us later).
        # SEPARATE semaphores per DMA (a single staged semaphore races:
        # fast rings can contribute several increments while slow rings
        # are still on the first sub-DMA).
        def w1(mt):
            # f1 row-block mt inside the packed sbuf tile
            if mt == 0:
                return fins[:, 0:128]
            return fins[:, 1152 + (mt - 1) * 128:1152 + mt * 128]

        def f2col(c):
            # f2^T column block c; bf16 bit patterns packed host-side
            # into the fp16 buffer (bitcast back on device)
            o = 128 + c if c < 1024 else 1536 + (c - 1024)
            return fins[:, o:o + 512].bitcast(mybir.dt.bfloat16)

        nc.sync.dma_start(fins[:, 0:640], fin[:, 0:640]).then_inc(in_a_sem, 16)
        nc.sync.dma_start(fins[:, 640:1152], fin[:, 640:1152]).then_inc(in_b_sem, 16)
        nc.sync.dma_start(fins[:, 1152:1536], fin[:, 1152:1536]).then_inc(in_c_sem, 16)
        nc.sync.dma_start(fins[:, 1536:2560], fin[:, 1536:2560]).then_inc(in_d_sem, 16)

        # chunk j = h*MT + mt  ->  psum tile pss[mt], out block (mt, h);
        # the first half-tile (mt0,h0) is further split into two 512-wide
        # exp/store chunks gated on mm0/mm1 individually, so the exp
        # stream starts one matmul earlier. echunks: (mt, col0, width).
        # echunk entries: (mt, out_col, psum_col, width)
        chunks = [(mt, h) for h in range(2) for mt in range(MT)]
        echunks = [(0, 0, 0, 512), (0, 512, 512, 512)] + [
            (mt, h * H, 0, H) for h in range(2) for mt in range(MT)
        ][1:]
        NCH = len(echunks)     # 9

        with nc.Block() as block:
            @block.scalar
            def _(scalar):
                # 1024-wide dummy reading (garbage) PSUM: loads the ACT
                # exp table during the input transfer in the SAME basic
                # block as the real exps (walrus' table-state pass resets
                # at BB boundaries and would re-load, +1.3us). It finishes
                # well before the first matmul writes ps0, and its garbage
                # output in ots chunk 0 is fully overwritten by the real
                # chunk-0 exp before any DMA reads it.
                nc.scalar.activation(ots[:, 0:H], ps0[:, :], EXPF)
                o = 0
                for j, (mt, c0, pc, w) in enumerate(echunks):
                    scalar.wait_ge(mm_sem, j + 1)
                    nc.scalar.activation(
                        ots[:, o:o + w],
                        pss[mt][:, pc:pc + w],
                        EXPF,
                    ).then_inc(act_sem)
                    o += w

            @block.tensor
            def _(tensor):
                tensor.wait_ge(in_a_sem, 16)
                for j, (mt, h) in enumerate(chunks):
                    ps = pss[mt]
                    if j == 1:
                        tensor.wait_ge(in_c_sem, 16)
                    if j == MT:
                        tensor.wait_ge(in_d_sem, 16)
                    if h == 1:
                        # reuse pss[mt]: wait until its h0 exps read out
                        # (echunk indices shift +1 from the mt0 split)
                        tensor.wait_ge(act_sem, mt + 2)
                    c = h * H
                    mma = nc.tensor.matmul(ps[:, 0:512], w1(mt), f2col(c),
                                           start=True, stop=True)
                    if j == 0:
                        mma.then_inc(mm_sem)
                        tensor.wait_ge(in_b_sem, 16)
                    nc.tensor.matmul(ps[:, 512:1024], w1(mt), f2col(c + 512),
                                     start=True, stop=True).then_inc(mm_sem)

            @block.sync
            def _(sync):
                # out_sem is incremented (DGE requires sync info) but never
                # waited on: the runtime epilogue (~8us all-engine sem
                # sweep after every program ends) hides the whole output
                # tail, so only sync's LAST INSTRUCTION time matters. The
                # trailing triggers pre-fire one exp early (waits below) so
                # the ~0.6us-per-DIRECT2D issue chain finishes before the
                # final exp retires; data safety comes from ring FIFO
                # order — each pre-fired chunk's descriptors queue behind
                # earlier chunks plus the junk padding transfer, so their
                # data is read only after their exp completes, with >=0.8us
                # margin that holds under engine-clock throttling (which
                # stretches the gating exps as much as the issue chain).
                waits = [1, 2, 3, 4, 5, 5, 6, 7, 7]
                o = 0
                for j, (mt, c0, pc, w) in enumerate(echunks):
                    sync.wait_ge(act_sem, waits[j])
                    sync.dma_start(
                        out[mt * 128:(mt + 1) * 128, c0:c0 + w],
                        ots[:, o:o + w],
                    ).then_inc(out_sem, 16)
                    o += w
                    if j == 6:
                        sync.dma_start(
                            junk[:, :], ots[:, 0:N2 // 2],
                        ).then_inc(out_sem, 16)

    # no explicit end-of-kernel semaphore cleanup: the NEFF's runtime
    # epilogue already sweeps every HW semaphore back to 0 on each engine
    # (observed as the anonymous $S[n]=0 EVENT_SEMAPHORE waves in traces),
    # so a re-execution of the loaded NEFF starts clean regardless

    _COMPILED = nc
    return _COMPILED


LAST_RESULTS = None


def _ensure_ntff_hook():
    """The agent image's `antenv` lacks `axon_hooks`; register the
    boot-shipped ctypes NTFF hook under that name so trace=True works."""
    import sys
    import types

    try:
        import antenv.axon_hooks  # noqa: F401
        return
    except ImportError:
        pass
    mod = types.ModuleType("antenv.axon_hooks")
    mod._hook = None

    def set_axon_ntff_profile_hook(hook):
        mod._hook = hook

    def get_axon_ntff_profile_hook():
        return mod._hook

    mod.set_axon_ntff_profile_hook = set_axon_ntff_profile_hook
    mod.get_axon_ntff_profile_hook = get_axon_ntff_profile_hook
    sys.modules["antenv.axon_hooks"] = mod
    import antenv

    antenv.axon_hooks = mod
    try:
        from trn_agent_boot.trn_boot import _ntff_profile_via_ctypes

        mod._hook = _ntff_profile_via_ctypes("/opt/axon/libaxon_pjrt.so")
    except Exception:
        pass
    # artifact upload needs bucket creds this container may not have;
    # the local NTFF -> perfetto pipeline doesn't depend on it
    import concourse.bass_utils as _bu

    _orig_upload = _bu.upload_artifacts

    def _safe_upload(tmpdir):
        try:
            return _orig_upload(tmpdir)
        except Exception:
            return tmpdir

    _bu.upload_artifacts = _safe_upload


def kernel(W1, W2, alpha, theta, gamma0, gamma1, gamma2, _profile=False):
    global LAST_RESULTS
    if _profile:
        _ensure_ntff_hook()
    F1, F2bits = _build_features(W1, W2, alpha, theta, gamma0, gamma1, gamma2)
    f1t = np.ascontiguousarray(F1.T)      # [D, N1] fp16
    # [D, N2] bf16 bit patterns reinterpreted as fp16 for the packed buffer
    f2t = np.ascontiguousarray(F2bits.T).view(np.float16)
    # packed per-core layout (must match the device's w1/f2col mapping):
    # [f1 block0 | f2[0:512] | f2[512:1024] | f1 blocks1-3 | f2[1024:2048]]
    in_maps = []
    for c in range(N_CORES):
        sh = f1t[:, c * ROWS:(c + 1) * ROWS]
        in_maps.append({
            "fin": np.ascontiguousarray(np.concatenate(
                [sh[:, 0:128], f2t[:, 0:512], f2t[:, 512:1024],
                 sh[:, 128:512], f2t[:, 1024:2048]], axis=1)),
        })
    nc = _get_nc()
    res = run_bass_kernel_spmd(nc, in_maps, list(range(N_CORES)), trace=_profile)
    LAST_RESULTS = res
    return np.concatenate(
        [res.results[c]["out"] for c in range(N_CORES)], axis=0
    ).astype(np.float32)
